# revision 30
# baseline (speedup 1.0000x reference)
"""Causal multi-head attention (B=2, T=2048, C=1024, H=16, d=64) on 8 trn2 cores.

Sharding: core i -> (batch b = i//4, head group g = i%4, 4 heads/core).
Data parallel over B, tensor parallel over heads; the out-proj partial sums
(contraction over this core's 256 channels) are reduced on the host during
the gather step, along with b_proj and the analytically-folded V bias.

All matmul operands are bfloat16 (full PE rate at any N; fp32 PSUM accum);
the y partials are DMA'd from PSUM in fp32 and summed on the host.

Device kernel works in [feature, token] (transposed) layout; ascending
q-blocks with stage-1 interleaved in chunks so the PE ramps with the DMA:
  stage 1 (chunk tc4): Q^T,K^T cols = (Wqk)^T x^T (+bias, 1/sqrt(d) folded
           into Wq host-side); V t-tiles 4*tc4..4*tc4+3 in natural layout.
           Chunk 0 upfront; chunk qb+1 emitted inside q-block qb's head loop.
  stage 2: S^T[j,q] = K_h^T.T @ Q_h^T  per head, causal tiles only
  stage 3: P^T = exp(S^T); the causal mask is applied AFTER exp as a 0/1
           multiply on the 128-wide diagonal block (off the S->P critical
           path; hidden behind the O-matmul lag)
  stage 4: outT[128,q] = [V_h | 1s(64)]^T.T @ P^T accumulated over j tiles
           (rows 64:128 = softmax denominator Z replicated 64x); O-matmuls
           are emitted a few groups behind their exp so the in-order PE
           never waits on ACT
  stage 5: att^T = outT[0:64] * reciprocal(outT[64:128])  (pure DVE)
  stage 6: y^T = Wp.T @ att^T -> fp32 DMA straight from PSUM; host sums.
"""

import numpy as np
import ml_dtypes

import concourse.bass as bass
import concourse.mybir as mybir
from concourse import bacc
import concourse.tile as tile
from concourse.bass_utils import run_bass_kernel_spmd

B, T, C, H, D = 2, 2048, 1024, 16, 64
NCORES = 8
HPC = 4            # heads per core
CS = HPC * D       # 256 channels per core (per Q/K/V block)
KT = C // 128      # 8 contraction tiles for the projections
NT = T // 128      # 16 token tiles of 128
QB = 512           # query block (psum bank width in fp32)
NQB = T // QB      # 4 query blocks

F32 = mybir.dt.float32
F16 = mybir.dt.float16
BF = mybir.dt.bfloat16
BF_NP = ml_dtypes.bfloat16

TRACE = False
LAST_RESULT = None


def _build_body(nc, tc, ctx, xT, wqk, wv, bqk, wp, masks, yT, yT2):
    AF = mybir.ActivationFunctionType

    persist = ctx.enter_context(tc.tile_pool(name="persist", bufs=1))

    wqk_sb = persist.tile([128, KT, 2 * CS], BF, tag="wqk", name="wqk")
    wv_sb = persist.tile([128, KT, CS], BF, tag="wv", name="wv")
    bqk_sb = persist.tile([128, 4], F32, tag="bqk", name="bqk")
    wp_sb = persist.tile([128, 2, C], BF, tag="wp", name="wp")
    mask_sb = persist.tile([128, 128], BF, tag="mask", name="mask_sb")
    qT_sb = [persist.tile([128, T], BF, tag=f"qT{i}", name=f"qT{i}") for i in range(2)]
    kT_sb = [persist.tile([128, T], BF, tag=f"kT{i}", name=f"kT{i}") for i in range(2)]
    # per j-tile: 4 heads x (64 ones cols | 64 V cols); the ones columns
    # replicate the softmax denominator into outT rows 0:64 (base partition 0
    # so reciprocal_approx_fast reads it correctly; custom DVE ops ignore a
    # nonzero input base partition)
    v_sb = [persist.tile([128, HPC, 2 * D], BF, tag=f"v{t}", name=f"v{t}") for t in range(NT)]
    attT_sb = [persist.tile([128, T], BF, tag=f"attT{i}", name=f"attT{i}") for i in range(2)]
    xT_sb = persist.tile([128, KT, T], BF, tag="xT", name="xT")

    widep = ctx.enter_context(tc.tile_pool(name="wide", bufs=2, space="PSUM"))
    oTp = ctx.enter_context(tc.tile_pool(name="oT", bufs=2, space="PSUM"))
    fillp = ctx.enter_context(tc.tile_pool(name="fill", bufs=1, space="PSUM"))
    pTp = ctx.enter_context(tc.tile_pool(name="pT", bufs=8))
    smallp = ctx.enter_context(tc.tile_pool(name="small", bufs=8))

    # ---------------- DMAs ----------------
    # each dma_start costs ~600ns of serial issue time on its engine, so the
    # per-k tiles are merged into wide transfers, issues are split across the
    # two HWDGE engines (SP + ACT), and the big x column-tail transfers are
    # issued from the gpsimd SWDGE queue AFTER its memsets so they don't
    # steal fabric bandwidth from the chunk-0 critical path
    xTr = xT.rearrange("(k p) t -> p k t", k=KT)
    wqkr = wqk.rearrange("(k p) c -> p k c", k=KT)
    nc.sync.dma_start(out=xT_sb[:, 0:1, 0:QB], in_=xTr[:, 0:1, 0:QB])
    nc.scalar.dma_start(out=wqk_sb[:, 0:1, :], in_=wqkr[:, 0:1, :])
    nc.sync.dma_start(out=xT_sb[:, 1:2, 0:QB], in_=xTr[:, 1:2, 0:QB])
    nc.scalar.dma_start(out=wqk_sb[:, 1:2, :], in_=wqkr[:, 1:2, :])
    nc.sync.dma_start(out=xT_sb[:, 2:4, 0:QB], in_=xTr[:, 2:4, 0:QB])
    nc.scalar.dma_start(out=wqk_sb[:, 2:4, :], in_=wqkr[:, 2:4, :])
    nc.sync.dma_start(out=xT_sb[:, 4:KT, 0:QB], in_=xTr[:, 4:KT, 0:QB])
    nc.scalar.dma_start(out=wqk_sb[:, 4:KT, :], in_=wqkr[:, 4:KT, :])
    nc.scalar.dma_start(out=mask_sb[:, :], in_=masks[:, :])
    nc.scalar.dma_start(out=bqk_sb[:, :], in_=bqk.rearrange("(c p) one -> p (c one)", c=4))
    nc.scalar.dma_start(out=wv_sb[:, :, :], in_=wv.rearrange("(k p) c -> p k c", k=KT))
    for t in range(NT):
        nc.gpsimd.memset(v_sb[t][:, :, :], 1.0)
    for tc4 in range(1, 4):
        nc.gpsimd.dma_start(
            out=xT_sb[:, :, tc4 * QB:(tc4 + 1) * QB],
            in_=xTr[:, :, tc4 * QB:(tc4 + 1) * QB],
        )
    nc.gpsimd.dma_start(out=wp_sb[:, :, :], in_=wp.rearrange("(k p) c -> p k c", k=2))

    # ---------------- filler work units ----------------
    # stage-1 chunks and stage-6 out-proj are chopped into ~0.2-0.4us PE
    # units and drained between attention groups, filling every ACT-lag
    # bubble while keeping the in-order PE busy
    def qk_pair_units(tc4, pair, pool):
        ref = {}
        units = []

        def bias_add(m):
            ps = ref["ps"]
            ct = pair[m]
            dst = qT_sb[ct] if ct < 2 else kT_sb[ct - 2]
            nc.vector.tensor_scalar_add(
                dst[:, tc4 * QB:(tc4 + 1) * QB], ps[:, m, :], bqk_sb[:, ct:ct + 1]
            )

        def kstep(k):
            def u():
                if k == 0:
                    ref["ps"] = pool.tile([128, 2, QB], F32, tag="sT", name="wps")
                ps = ref["ps"]
                for m, ct in enumerate(pair):
                    nc.tensor.matmul(
                        ps[:, m, :],
                        lhsT=wqk_sb[:, k, ct * 128:(ct + 1) * 128],
                        rhs=xT_sb[:, k, tc4 * QB:(tc4 + 1) * QB],
                        start=(k == 0),
                        stop=(k == KT - 1),
                    )
                if k == KT - 1:
                    bias_add(0)   # m0 drain starts a unit early
            return u

        units = [kstep(k) for k in range(KT)]
        units.append(lambda: bias_add(1))
        return units

    def v_pair_units(tc4, tp, pool):
        t0 = 4 * tc4 + 2 * tp
        ref = {}

        def copies(m):
            # one contiguous copy per head: strided/rearranged dst APs
            # mis-register their write deps (O-matmuls then race the copy)
            ps = ref["ps"]
            t = t0 + m
            for h in range(HPC):
                nc.vector.tensor_copy(
                    v_sb[t][:, h, D:2 * D], ps[:, m, h * D:(h + 1) * D]
                )

        def kstep(k):
            def u():
                if k == 0:
                    ref["ps"] = pool.tile([128, 2, QB], F32, tag="sT", name="wps")
                ps = ref["ps"]
                for m in range(2):
                    t = t0 + m
                    nc.tensor.matmul(
                        ps[:, m, 0:CS],
                        lhsT=xT_sb[:, k, t * 128:(t + 1) * 128],
                        rhs=wv_sb[:, k, :],
                        start=(k == 0),
                        stop=(k == KT - 1),
                    )
                if k == KT - 1:
                    copies(0)   # t0 drain starts a unit early
            return u

        units = [kstep(k) for k in range(KT)]
        units.append(lambda: copies(1))
        return units

    def chunk_units(tc4, pool):
        units = []
        for pair in ((0, 2), (1, 3)):
            units += qk_pair_units(tc4, pair, pool)
        for tp in range(2):
            units += v_pair_units(tc4, tp, pool)
        return units

    def stage6_units(sqb, pool, kcs=(0, 1), out_t=None, alt_copy=False):
        # each et is two queue entries: the PE chain, then the drain (copy +
        # DMA) a group later — so the next chain's psum WAR never stalls the
        # PE on the previous drain's vector CAST
        ot = out_t if out_t is not None else yT
        units = []
        for et in range(C // 128):
            ref = {}

            def chain(et=et, ref=ref):
                # same tag/shape as the stage-1 chain tiles so fillp stays at
                # one 2-bank slot; the chain only uses the first bank
                yps_w = pool.tile([128, 2, QB], F32, tag="sT", name="yps")
                ref["yps"] = yps_w[:, 0, :]
                for j, kc in enumerate(kcs):
                    nc.tensor.matmul(
                        ref["yps"],
                        lhsT=wp_sb[:, kc, et * 128:(et + 1) * 128],
                        rhs=attT_sb[kc][:, sqb * QB:(sqb + 1) * QB],
                        start=(j == 0),
                        stop=(j == len(kcs) - 1),
                    )

            def drain(et=et, ref=ref):
                # DMA straight from PSUM: no staging copy, so the psum WAR
                # never convoys behind the in-order vector queue
                nc.sync.dma_start(
                    out=ot[et * 128:(et + 1) * 128, sqb * QB:(sqb + 1) * QB],
                    in_=ref["yps"],
                )
            units.append(chain)
            units.append(drain)
        return units

    # ---------------- attention piece emitters ----------------
    def emit_S(h, qb, grp, sT):
        ktile = kT_sb[h // 2]
        qtile = qT_sb[h // 2]
        po = (h % 2) * D
        diag = grp >= 2 * (qb + 1) - 2
        for m in range(2):
            jt = grp * 2 + m
            c0 = 128 * (jt - 4 * qb) if diag else 0
            nc.tensor.matmul(
                sT[:, m, c0:QB],
                lhsT=ktile[po:po + D, jt * 128:(jt + 1) * 128],
                rhs=qtile[po:po + D, qb * QB + c0:(qb + 1) * QB],
                start=True,
                stop=True,
            )

    def emit_exp(h, qb, grp, sT, pT):
        diag = grp >= 2 * (qb + 1) - 2
        if diag:
            for m in range(2):
                c0 = 128 * (grp * 2 + m - 4 * qb)
                nc.scalar.activation(pT[:, m, c0:QB], sT[:, m, c0:QB], AF.Exp)
                # 0/1 causal mask on the diagonal 128-block, after exp so the
                # S->P critical path has no DVE hop; hidden by the O lag
                nc.vector.tensor_mul(
                    pT[:, m, c0:c0 + 128], pT[:, m, c0:c0 + 128], mask_sb[:, :]
                )
        else:
            nc.scalar.activation(pT[:, :, :], sT[:, :, :], AF.Exp)

    def emit_O(h, qb, grp, pT, oT):
        njt = 4 * (qb + 1)
        diag = grp >= 2 * (qb + 1) - 2
        for m in range(2):
            jt = grp * 2 + m
            c0 = 128 * (jt - 4 * qb) if diag else 0
            nc.tensor.matmul(
                oT[:, c0:QB],
                lhsT=v_sb[jt][:, h, :],
                rhs=pT[:, m, c0:QB],
                start=(jt == 0),
                stop=(jt == njt - 1),
            )

    def emit_norm(h, qb, oT):
        po = (h % 2) * D
        rz = smallp.tile([D, QB], F32, tag="rz", name="rz")
        nc.vector.reciprocal_approx_fast(out=rz[:, :], in_=oT[0:D, :])
        nc.vector.tensor_mul(
            attT_sb[h // 2][po:po + D, qb * QB:(qb + 1) * QB],
            oT[D:2 * D, :],
            rz[:, :],
        )

    def emit_stage6(sqb):
        # y^T[e, sqb] = Wp.T @ att^T[:, sqb] -> fp16 out
        for et in range(C // 128):
            yps_t = oTp.tile([128, QB], F32, tag="oT", name="yps")
            for kc in range(2):
                nc.tensor.matmul(
                    yps_t[:, :],
                    lhsT=wp_sb[:, kc, et * 128:(et + 1) * 128],
                    rhs=attT_sb[kc][:, sqb * QB:(sqb + 1) * QB],
                    start=(kc == 0),
                    stop=(kc == 1),
                )
            ys = ysp.tile([128, QB], F16, tag="ys", name="ys")
            nc.any.tensor_copy(ys[:, :], yps_t[:, :])
            nc.sync.dma_start(
                out=yT[et * 128:(et + 1) * 128, sqb * QB:(sqb + 1) * QB],
                in_=ys[:, :],
            )

    # ---------------- main schedule ----------------
    # chunk 0 upfront (the DMA ramp); spread its 4 psum tiles over both pools
    for un in qk_pair_units(0, (0, 2), widep):
        un()
    for un in qk_pair_units(0, (1, 3), widep):
        un()
    for un in v_pair_units(0, 0, fillp):
        un()
    for un in v_pair_units(0, 1, fillp):
        un()

    dq = []   # deadline units: chunk(qb+1) must finish within qb
    fq = []   # free units: stage6 partials, drained opportunistically

    def drain(groups_left):
        n = -(-len(dq) // groups_left) if groups_left > 0 else len(dq)
        took = 0
        for _ in range(min(n, len(dq))):
            dq.pop(0)()
            took += 1
        if took == 0 and fq:
            fq.pop(0)()

    for qb in range(NQB):
        ngr = 2 * (qb + 1)
        if qb >= 1:
            fq.extend(stage6_units(qb - 1, [fillp]))
        if qb < NQB - 1:
            dq.extend(chunk_units(qb + 1, fillp))
        g_left = HPC * ngr
        # heads run in pairs, groups round-robin across the pair: exp(h,g)
        # gets two heads' worth of PE time before its O-matmuls are due, so
        # the single ACT stream never gates the in-order PE
        for hp in ((0, 1), (2, 3)):
            oTs = {h: oTp.tile([128, QB], F32, tag="oT", name="oT") for h in hp}
            pend = []
            for grp in range(ngr):
                for h in hp:
                    sT = widep.tile([128, 2, QB], F32, tag="sT", name="sT")
                    emit_S(h, qb, grp, sT)
                    pT = pTp.tile([128, 2, QB], BF, tag="pT", name="pT")
                    emit_exp(h, qb, grp, sT, pT)
                    drain(g_left)
                    g_left -= 1
                    state["tg_left"] -= 1
                    pend.append((h, grp, pT))
                    if len(pend) > 3:
                        h0_, g0, p0 = pend.pop(0)
                        emit_O(h0_, qb, g0, p0, oTs[h0_])
            if dq:
                dq.pop(0)()
            elif fq:
                fq.pop(0)()
            for h0_, g0, p0 in pend:
                emit_O(h0_, qb, g0, p0, oTs[h0_])
            for h in hp:
                emit_norm(h, qb, oTs[h])
            if qb == NQB - 1 and hp == (0, 1):
                # last q-block: its kc0 out-proj half only needs heads 0,1 —
                # feed it to the tail-end groups as free filler
                fq.extend(stage6_units(NQB - 1, [fillp], kcs=(0,)))
        while dq:
            dq.pop(0)()
    while fq:
        fq.pop(0)()
    # tail: the kc1 half of the last q-block goes to yT2, summed on the host
    for un in stage6_units(NQB - 1, fillp, kcs=(1,), out_t=yT2, alt_copy=True):
        un()


def build_nc():
    from contextlib import ExitStack

    nc = bacc.Bacc("TRN2", target_bir_lowering=False)
    xT = nc.dram_tensor("xT", [C, T], BF, kind="ExternalInput")
    wqk = nc.dram_tensor("wqk", [C, 2 * CS], BF, kind="ExternalInput")
    wv = nc.dram_tensor("wv", [C, CS], BF, kind="ExternalInput")
    bqk = nc.dram_tensor("bqk", [2 * CS, 1], F32, kind="ExternalInput")
    wp = nc.dram_tensor("wp", [CS, C], BF, kind="ExternalInput")
    masks = nc.dram_tensor("masks", [128, 128], BF, kind="ExternalInput")
    yT = nc.dram_tensor("yT", [C, T], F32, kind="ExternalOutput")
    yT2 = nc.dram_tensor("yT2", [C, T], F32, kind="ExternalOutput")
    with tile.TileContext(nc) as tc:
        with nc.allow_low_precision(reason="bf16 matmul inputs; accumulation stays fp32 in PSUM"):
            with ExitStack() as ctx:
                _build_body(nc, tc, ctx, xT, wqk, wv, bqk, wp, masks, yT, yT2)
    nc.compile()
    return nc


def make_mask01():
    r = np.arange(128)[:, None]
    c = np.arange(128)[None, :]
    return np.where(r <= c, 1.0, 0.0).astype(BF_NP)


def make_in_maps(x, W_qkv, b_qkv, W_proj):
    scale = np.float32(1.0 / np.sqrt(D))
    mask_h = make_mask01()
    in_maps = []
    for i in range(NCORES):
        b, g = divmod(i, HPC)
        cs0 = g * CS
        wq = W_qkv[:, cs0:cs0 + CS] * scale
        wk = W_qkv[:, C + cs0:C + cs0 + CS]
        bq = b_qkv[cs0:cs0 + CS] * scale
        bk = b_qkv[C + cs0:C + cs0 + CS]
        in_maps.append({
            "xT": np.ascontiguousarray(x[b].T).astype(BF_NP),
            "wqk": np.concatenate([wq, wk], axis=1).astype(BF_NP),
            "wv": np.ascontiguousarray(W_qkv[:, 2 * C + cs0:2 * C + cs0 + CS]).astype(BF_NP),
            "bqk": np.concatenate([bq, bk])[:, None].astype(np.float32),
            "wp": np.ascontiguousarray(W_proj[cs0:cs0 + CS, :]).astype(BF_NP),
            "masks": mask_h,
        })
    return in_maps


_NC_CACHE = None


def _get_nc():
    global _NC_CACHE
    if _NC_CACHE is None:
        _NC_CACHE = build_nc()
    return _NC_CACHE


def gather(results, b_qkv, W_proj, b_proj):
    Y = np.zeros((B, T, C), np.float32)
    for i in range(NCORES):
        Y[i // HPC] += results[i]["yT"].T.astype(np.float32)
        qb3 = (NQB - 1) * QB
        Y[i // HPC][qb3:] += results[i]["yT2"].T[qb3:].astype(np.float32)
    Y += (b_qkv[2 * C:].astype(np.float32) @ W_proj.astype(np.float32)
          + b_proj.astype(np.float32))[None, None, :]
    return Y


def kernel(x, W_qkv, b_qkv, W_proj, b_proj):
    global LAST_RESULT
    x = np.asarray(x, np.float32)
    W_qkv = np.asarray(W_qkv, np.float32)
    b_qkv = np.asarray(b_qkv, np.float32)
    W_proj = np.asarray(W_proj, np.float32)
    b_proj = np.asarray(b_proj, np.float32)

    nc = _get_nc()
    in_maps = make_in_maps(x, W_qkv, b_qkv, W_proj)
    res = run_bass_kernel_spmd(nc, in_maps, list(range(NCORES)), trace=TRACE)
    LAST_RESULT = res
    if TRACE and res.exec_time_ns is not None:
        print(f"HW exec time: {res.exec_time_ns} ns")
    return gather(res.results, b_qkv, W_proj, b_proj)
